# revision 16
# baseline (speedup 1.0000x reference)
"""TRN2 Bass kernel for nn_CosFreqEncoding via reassociation:
out = x @ (W.T @ cos_basis) / max.

Strategy: shard the OUTPUT COLUMNS across the 8 cores. Core i computes
M_i = (W.T @ cos)[:, i*256:(i+1)*256] from the full W and its cos column
slice (phase A, 1/8 of the M GEMM, no redundancy), then
outT_i = M_i.T-oriented GEMM against the full x.T (phase B), covering
out[:, i*256:(i+1)*256] for ALL 4096 batch rows. Total PE rows drop 28%
vs the two-GEMM data-parallel form (201k vs 279k) and no tensor-sized
collective is needed -- only the scalar AllReduce(max) for normalization.

Layouts (all natural, no host transposes except x.T):
  A: psumA[l, c]  += W[f, l-slice].T @ cosS[f, c]      (both f-major)
  B: psumB[c, m]  += Msb[l, c-slice].T @ xT[l, m]      (both l-major)

bf16 operands, f32 accumulate. GpSimd only triggers collectives and does
two tiny cross-lane reduces (no SWDGE DMAs -> short epilogue drain).

Self-contained: hardcodes shapes from the problem spec.
"""
import ml_dtypes
import numpy as np

import concourse.bass as bass
import concourse.bacc as bacc
import concourse.mybir as mybir
import concourse.tile as tile
import concourse.bass_utils as bass_utils

N_CORES = 8
B, L, F = 4096, 2048, 2074
FP = 2176               # F padded to 17 full 128-tiles
CS = L // N_CORES       # 256 output columns per core
LT = L // 128           # 16 l-tiles
FT = FP // 128          # 17 f-tiles (phase A contraction)
F32 = mybir.dt.float32
BF16 = mybir.dt.bfloat16


def _armax(nc, sp, dp, vm_slice, tag, dmaq):
    """Local max of vm_slice -> scalar -> AllReduce(max); returns dram out."""
    g = sp.tile([1, 1], F32, name=f"g_{tag}")
    nc.gpsimd.reduce_max(g[:], vm_slice, axis=mybir.AxisListType.XYZWC)
    cc_in = dp.tile([1], F32, name=f"ccin_{tag}")
    cc_out = dp.tile([1], F32, name=f"ccout_{tag}")
    dmaq.dma_start(cc_in[:], g[:, 0])
    nc.gpsimd.collective_compute(
        "AllReduce", mybir.AluOpType.max,
        replica_groups=[list(range(N_CORES))],
        ins=[cc_in[:]], outs=[cc_out[:]])
    return cc_out


def _emit(nc, tc, xT, Wb, cosS, out):
    with (
        tc.tile_pool(name="wp", bufs=2) as wp,
        tc.tile_pool(name="cp", bufs=1) as cp,
        tc.tile_pool(name="mp", bufs=1) as mp,
        tc.tile_pool(name="xp", bufs=6) as xp,
        tc.tile_pool(name="op", bufs=1) as op,
        tc.tile_pool(name="sp", bufs=1) as sp,
        tc.tile_pool(name="ps", bufs=8, space="PSUM") as ps,
        tc.tile_pool(name="dp", bufs=1, space="DRAM") as dp,
    ):
        qs = [nc.sync, nc.scalar]

        # Warmup AllReduce: absorbs the cross-core bootstrap barrier and the
        # first-collective setup cost.
        wz = sp.tile([1, 1], F32, name="warm_z")
        nc.vector.memset(wz[:], 0.0)
        warm_in = dp.tile([1], F32, name="warm_in")
        warm_out = dp.tile([1], F32, name="warm_out")
        nc.sync.dma_start(warm_in[:], wz[:, 0])
        nc.gpsimd.collective_compute(
            "AllReduce", mybir.AluOpType.max,
            replica_groups=[list(range(N_CORES))],
            ins=[warm_in[:]], outs=[warm_out[:]])

        # ---- Phase A: M_i[2048, 256] = W.T @ cosS, two halves of 8 l-tiles.
        # cos slice resident in SBUF; W streamed per half as a few large
        # per-partition-contiguous DMAs. First-needed pieces (cos ft0 on
        # scalar, W ft0 on sync) issue ahead of the bulk so the first matmul
        # fires early. Each accumulation owns a full PSUM bank (matmul
        # start=True clears the whole bank).
        cbig = cp.tile([128, FT * CS], BF16, name="cosr")
        msb = [mp.tile([128, CS], BF16, name=f"msb{lt}") for lt in range(LT)]
        wbigs = [wp.tile([128, FT * 8 * 128], BF16, tag="w", name=f"wbig{h}")
                 for h in range(2)]
        # h0 feed, interleaved so each f-tile lands just ahead of its matmuls
        nc.scalar.dma_start(cbig[:, 0:CS], cosS[:, 0:CS])
        nc.sync.dma_start(wbigs[0][:, 0:1024], Wb[0, :, 0:1024])
        nc.scalar.dma_start(wbigs[0][:, 1024:2048], Wb[0, :, 1024:2048])
        nc.sync.dma_start(cbig[:, CS:5 * CS], cosS[:, CS:5 * CS])
        nc.sync.dma_start(wbigs[0][:, 2048:4096], Wb[0, :, 2048:4096])
        nc.scalar.dma_start(cbig[:, 5 * CS:FT * CS], cosS[:, 5 * CS:FT * CS])
        nc.scalar.dma_start(wbigs[0][:, 4096:6144], Wb[0, :, 4096:6144])
        nc.sync.dma_start(wbigs[0][:, 6144:9216], Wb[0, :, 6144:9216])
        nc.scalar.dma_start(wbigs[0][:, 9216:13312], Wb[0, :, 9216:13312])
        nc.sync.dma_start(wbigs[0][:, 13312:FT * 1024], Wb[0, :, 13312:FT * 1024])
        WCH = [(0, 3), (3, 7), (7, 11), (11, 14), (14, 17)]
        for h in range(2):
            pa = [ps.tile([128, 512], F32, tag="ps", name=f"pa{h}_{k}")
                  for k in range(8)]
            wbig = wbigs[h]
            if h == 1:
                for n, (f0, f1) in enumerate(WCH):
                    qs[n % 2].dma_start(wbig[:, f0 * 1024:f1 * 1024],
                                        Wb[h, :, f0 * 1024:f1 * 1024])
            for ft in range(FT):
                for lk in range(8):
                    nc.tensor.matmul(
                        pa[lk][:, 0:CS],
                        wbig[:, ft * 1024 + lk * 128:ft * 1024 + (lk + 1) * 128],
                        cbig[:, ft * CS:(ft + 1) * CS],
                        start=(ft == 0), stop=(ft == FT - 1))
            for k in range(8):
                if k % 2 == 0:
                    nc.vector.tensor_copy(msb[h * 8 + k][:], pa[k][:, 0:CS])
                else:
                    nc.scalar.copy(msb[h * 8 + k][:], pa[k][:, 0:CS])

        # ---- Phase B: outT[256, 4096] = M_i.T @ x, two passes of 4 m-chunks
        ot = [op.tile([128, B], F32, name=f"ot{ct}") for ct in range(2)]
        vmaxes = sp.tile([128, 16], F32)
        cco2 = dp.tile([1], F32, name="cco2")
        for p in range(2):
            pb = [ps.tile([128, 512], F32, tag="ps", name=f"pb{p}_{j}")
                  for j in range(8)]
            for lt in range(LT):
                xt = xp.tile([128, 2048], BF16, tag="x")
                qs[lt % 2].dma_start(xt[:], xT[lt, :, p * 2048:(p + 1) * 2048])
                for ct in range(2):
                    lhsT = msb[lt][:, ct * 128:(ct + 1) * 128]
                    for mc in range(4):
                        nc.tensor.matmul(
                            pb[ct * 4 + mc][:], lhsT,
                            xt[:, mc * 512:(mc + 1) * 512],
                            start=(lt == 0), stop=(lt == LT - 1))
            # flush: per-bank reduce (DVE) + copy (Activation) interleaved so
            # each PSUM bank frees in ~1us for the next pass; a single final
            # AllReduce covers all 16 maxes -- with the warmup having
            # absorbed the bootstrap, an intermediate stage only lengthens
            # the serialized cc-stream chain
            for j in range(8):
                ct, mc = j // 4, j % 4
                k = p * 4 + mc
                nc.vector.reduce_max(vmaxes[:, p * 8 + j:p * 8 + j + 1],
                                     pb[j][:], axis=mybir.AxisListType.X)
                nc.scalar.copy(ot[ct][:, k * 512:(k + 1) * 512], pb[j][:])
            if p == 1:
                g2 = sp.tile([1, 1], F32, name="g_s2")
                nc.gpsimd.reduce_max(g2[:], vmaxes[:],
                                     axis=mybir.AxisListType.XYZWC)
                cc_in2 = dp.tile([1], F32, name="ccin2")
                nc.sync.dma_start(cc_in2[:], g2[:, 0])
                nc.gpsimd.collective_compute(
                    "AllReduce", mybir.AluOpType.max,
                    replica_groups=[list(range(N_CORES))],
                    ins=[cc_in2[:]], outs=[cco2[:]])

        # broadcast the global max to all partitions and invert
        gbc = sp.tile([128, 1], F32)
        nc.sync.dma_start(gbc[:], cco2[:].partition_broadcast(128))
        rbc = sp.tile([128, 1], F32)
        nc.vector.reciprocal(rbc[:], gbc[:])

        # scale (f32 -> bf16) + store in [128, 1024] chunks; muls split
        # DVE/Activation (5/3 balances their rates), stores all on sync
        ots = [op.tile([128, B], BF16, name=f"ots{ct}") for ct in range(2)]
        order = [(ct, kk) for ct in range(2) for kk in range(4)]
        for n, (ct, kk) in enumerate(order):
            sl = slice(kk * 1024, (kk + 1) * 1024)
            if n in (1, 4, 6):
                nc.scalar.mul(ots[ct][:, sl], ot[ct][:, sl], rbc[:, 0:1])
            else:
                nc.vector.tensor_scalar_mul(ots[ct][:, sl], ot[ct][:, sl],
                                            rbc[:, 0:1])
            nc.sync.dma_start(
                out[ct * 128:(ct + 1) * 128, sl], ots[ct][:, sl])


def _build():
    nc = bacc.Bacc("TRN2", target_bir_lowering=False, debug=False,
                   num_devices=N_CORES)
    xT = nc.dram_tensor("xT", [LT, 128, B], BF16, kind="ExternalInput")
    # Wb[h, p, ft*8*128 + lk*128 + b] = Wp[ft*128+p, (h*8+lk)*128+b]
    Wb = nc.dram_tensor("Wb", [2, 128, FT * 8 * 128], BF16,
                        kind="ExternalInput")
    # cosS[p, ft*CS + c] = cosp[ft*128+p, core_lo + c]
    cosS = nc.dram_tensor("cosS", [128, FT * CS], BF16, kind="ExternalInput")
    out = nc.dram_tensor("out", [CS, B], BF16, kind="ExternalOutput")
    with tile.TileContext(nc) as tc:
        _emit(nc, tc, xT, Wb, cosS, out)
    nc.compile()
    return nc


_cached_nc = None


def _get_nc():
    global _cached_nc
    if _cached_nc is None:
        _cached_nc = _build()
    return _cached_nc


def _bf16(a: np.ndarray) -> np.ndarray:
    return np.ascontiguousarray(a, dtype=np.float32).astype(ml_dtypes.bfloat16)


def _prep_inputs(x, W, cos_basis):
    x = np.ascontiguousarray(x, dtype=np.float32)
    W = np.ascontiguousarray(W, dtype=np.float32)
    cos = np.ascontiguousarray(cos_basis, dtype=np.float32)
    Wp = np.zeros((FP, L), dtype=np.float32)
    Wp[:F] = W
    cosp = np.zeros((FP, L), dtype=np.float32)
    cosp[:F] = cos
    # Wb[h, p, (ft, lk, b)] = Wp[ft*128+p, (h*8+lk)*128+b]
    W4 = Wp.reshape(FT, 128, LT, 128)
    Wb = _bf16(np.stack([
        np.ascontiguousarray(
            W4[:, :, h * 8:(h + 1) * 8, :].transpose(1, 0, 2, 3)
        ).reshape(128, FT * 8 * 128)
        for h in range(2)]))
    xTf = _bf16(np.ascontiguousarray(x.T).reshape(LT, 128, B))
    # cosS[p, (ft, c)] = cosp[ft*128+p, i*CS+c]
    cosSs = [_bf16(np.ascontiguousarray(
        cosp[:, i * CS:(i + 1) * CS].reshape(FT, 128, CS).transpose(1, 0, 2)
    ).reshape(128, FT * CS)) for i in range(N_CORES)]
    return xTf, Wb, cosSs


def kernel(x, W, cos_basis, _trace=False, _trace_kwargs=None):
    xTf, Wb, cosSs = _prep_inputs(x, W, cos_basis)
    nc = _get_nc()
    in_maps = [{"xT": xTf, "Wb": Wb, "cosS": cosSs[i]}
               for i in range(N_CORES)]
    res = bass_utils.run_bass_kernel_spmd(
        nc, in_maps, core_ids=list(range(N_CORES)), trace=_trace,
        **(_trace_kwargs or {}))
    full = np.empty((B, L), dtype=np.float32)
    for i in range(N_CORES):
        full[:, i * CS:(i + 1) * CS] = res.results[i]["out"].astype(np.float32).T
    if _trace:
        kernel.last_result = res
    return full
